# revision 4
# baseline (speedup 1.0000x reference)
"""NonLocalBlock (B=4, C=64, Ci=32, H=W=64) on 8 TRN2 NeuronCores.

Sharding: data-parallel over batch (4 pairs of cores); within each pair
the query dimension n of the NxN score matrix is split in half.
Softmax runs over n (dim=1), so each core computes partial softmax
denominators S[m] over its n-half; tiny pairwise AllReduces ([128 x g]
f32) produce the full denominators. Everything else is local: each
core produces z[:, n_half] and the host concatenates.

Per core (b = core//2, h = core%2):
  theta = theta_w @ supp[:, nh] + theta_b           [32, 2048]  bf16
  phi   = phi_w @ ref + phi_b                       [32, 4096]  bf16
  fT    = phi_tile^T @ theta   (per m-tile of 128)  [128, 2048] PSUM f32
  expT  = exp(fT)  (ACT)                            bf16 SBUF
  S     = row-sum of expT (DVE reduce), AllReduce_pair
  wgT   = ref_aug^T @ (w_w@g_w | w_w@g_b)^T         [128, 64] per m-tile
  wgT'  = wgT * (1/S)   (softmax scale + final 1x1 conv folded into g)
  z     = sum_mt wgT'^T @ expT  (col-tiled PSUM accum, [128 x 1024]:
          partitions 0:64 = n-cols 0:1024, 64:128 = n-cols 1024:2048)
  out   = supp[:, nh] + z + w_b   (DVE, DMA'd as repacked [128, 1024])
"""

import numpy as np

B, C, CI, H, W = 4, 64, 32, 64, 64
N = H * W            # 4096
NLOC = N // 2        # 2048 n-columns per core
NCORES = 8
MTP = 128            # m-tile partition size
NMT = N // MTP       # 32 m-tiles
GROUP_SIZES = [6, 6, 6, 6, 4, 4]
CK = 512             # matmul moving-dim chunk

REPLICA_GROUPS = [[0, 1], [2, 3], [4, 5], [6, 7]]

_cache = {}


def _build():
    import concourse.bacc as bacc
    import concourse.tile as tile
    from concourse import mybir

    f32 = mybir.dt.float32
    bf16 = mybir.dt.bfloat16
    AF = mybir.ActivationFunctionType
    ALU = mybir.AluOpType
    AX = mybir.AxisListType

    nc = bacc.Bacc(None, target_bir_lowering=False, debug=False)

    supp_rp = nc.dram_tensor("supp_rp", [MTP, NLOC // 2], f32, kind="ExternalInput")
    supp_b = nc.dram_tensor("supp_b", [C, NLOC], bf16, kind="ExternalInput")
    ref_aug = nc.dram_tensor("ref_aug", [C + 1, N], bf16, kind="ExternalInput")
    theta_wT = nc.dram_tensor("theta_wT", [C, CI], bf16, kind="ExternalInput")
    theta_bc = nc.dram_tensor("theta_bc", [CI, 1], f32, kind="ExternalInput")
    phi_wT = nc.dram_tensor("phi_wT", [C, CI], bf16, kind="ExternalInput")
    phi_bc = nc.dram_tensor("phi_bc", [CI, 1], f32, kind="ExternalInput")
    wg_aug = nc.dram_tensor("wg_aug", [C + 1, C], bf16, kind="ExternalInput")
    w_bc2 = nc.dram_tensor("w_bc2", [MTP, 1], f32, kind="ExternalInput")
    out = nc.dram_tensor("out", [MTP, NLOC // 2], f32, kind="ExternalOutput")

    NG = len(GROUP_SIZES)
    assert sum(GROUP_SIZES) == NMT
    group_of = []
    for g, gs in enumerate(GROUP_SIZES):
        group_of += [g] * gs
    group_start = [sum(GROUP_SIZES[:g]) for g in range(NG)]

    with tile.TileContext(nc) as tc:
        from contextlib import ExitStack

        with ExitStack() as ctx:
            sing = ctx.enter_context(tc.tile_pool(name="sing", bufs=1))
            spool = ctx.enter_context(tc.tile_pool(name="spool", bufs=NG))
            epool = ctx.enter_context(tc.tile_pool(name="expT", bufs=NMT))
            dpool = ctx.enter_context(
                tc.tile_pool(name="dram", bufs=NG, space="DRAM")
            )
            outp = ctx.enter_context(tc.tile_pool(name="outp", bufs=2))
            # PSUM budget (8 banks x 2KB): ftp 3x[128,1024]f32 = 6 banks for
            # the whole kernel; psA (projections, 2 banks) closes early and
            # hands its banks to wgtp, which closes and hands them to zpp.
            ftp = ctx.enter_context(tc.tile_pool(name="ftp", bufs=3, space="PSUM"))

            # ---------------- loads ----------------
            # big tensors on separate queues so they stream in parallel
            refa = sing.tile([C + 1, N], bf16, tag="refa")
            nc.sync.dma_start(out=refa, in_=ref_aug[:, :])
            supp_bf = sing.tile([C, NLOC], bf16, tag="suppbf")
            nc.scalar.dma_start(out=supp_bf, in_=supp_b[:, :])
            supp_t = sing.tile([MTP, NLOC // 2], f32, tag="supp")
            nc.scalar.dma_start(out=supp_t, in_=supp_rp[:, :])
            tw = sing.tile([C, CI], bf16, tag="tw")
            nc.gpsimd.dma_start(out=tw, in_=theta_wT[:, :])
            tb = sing.tile([CI, 1], f32, tag="tb")
            nc.gpsimd.dma_start(out=tb, in_=theta_bc[:, :])
            pw = sing.tile([C, CI], bf16, tag="pw")
            nc.gpsimd.dma_start(out=pw, in_=phi_wT[:, :])
            pb = sing.tile([CI, 1], f32, tag="pb")
            nc.gpsimd.dma_start(out=pb, in_=phi_bc[:, :])
            wga = sing.tile([C + 1, C], bf16, tag="wga")
            nc.gpsimd.dma_start(out=wga, in_=wg_aug[:, :])
            wb = sing.tile([MTP, 1], f32, tag="wb")
            nc.gpsimd.dma_start(out=wb, in_=w_bc2[:, :])

            theta_t = sing.tile([CI, NLOC], bf16, tag="theta")
            phi_t = sing.tile([CI, N], bf16, tag="phi")
            wgt_raw = sing.tile([MTP, NMT * C], f32, tag="wgtraw")
            wgt_b16 = sing.tile([MTP, NMT * C], bf16, tag="wgtb16")

            # -------- projections (also the PE warm-up burst) --------
            psA_ctx = ExitStack()
            psA = psA_ctx.enter_context(
                tc.tile_pool(name="psA", bufs=2, space="PSUM")
            )
            for j in range(NLOC // CK):
                ps = psA.tile([CI, CK], f32, tag="projps")
                nc.tensor.matmul(
                    ps,
                    lhsT=tw[:, :],
                    rhs=supp_bf[:, j * CK : (j + 1) * CK],
                    start=True,
                    stop=True,
                )
                nc.vector.tensor_scalar_add(
                    theta_t[:, j * CK : (j + 1) * CK], ps, tb[:, :]
                )
            for j in range(N // CK):
                ps = psA.tile([CI, CK], f32, tag="projps", name=f"phi_ps{j}")
                nc.tensor.matmul(
                    ps,
                    lhsT=pw[:, :],
                    rhs=refa[0:C, j * CK : (j + 1) * CK],
                    start=True,
                    stop=True,
                )
                nc.vector.tensor_scalar_add(
                    phi_t[:, j * CK : (j + 1) * CK], ps, pb[:, :]
                )
            psA_ctx.close()

            wgt_ctx = ExitStack()
            wgtp = wgt_ctx.enter_context(
                tc.tile_pool(name="wgtp", bufs=2, space="PSUM")
            )

            state = {"z": None}
            wgt_queue = list(range(NMT))
            ets = [None] * NMT
            srecs = [None] * NG

            def emit_wgt(mt):
                ps = wgtp.tile([MTP, C], f32, tag="wgtps")
                nc.tensor.matmul(
                    ps,
                    lhsT=refa[:, mt * MTP : (mt + 1) * MTP],
                    rhs=wga[:, :],
                    start=True,
                    stop=True,
                )
                nc.vector.tensor_copy(wgt_raw[:, mt * C : (mt + 1) * C], ps)

            def emit_c(mt):
                g = group_of[mt]
                tl = mt - group_start[g]
                nc.vector.tensor_scalar_mul(
                    wgt_b16[:, mt * C : (mt + 1) * C],
                    wgt_raw[:, mt * C : (mt + 1) * C],
                    srecs[g][:, tl : tl + 1],
                )
                # col-tiled z: partitions 0:64 accumulate n 0:1024,
                # partitions 64:128 accumulate n 1024:2048. Interleave the
                # two column-halves so they run concurrently on the PE.
                z = state["z"]
                w = wgt_b16[:, mt * C : (mt + 1) * C]
                e = ets[mt]
                for jj in range(2):
                    for ph in range(2):
                        nc.tensor.matmul(
                            z[ph * C : (ph + 1) * C, jj * CK : (jj + 1) * CK],
                            lhsT=w,
                            rhs=e[:, ph * 1024 + jj * CK : ph * 1024 + (jj + 1) * CK],
                            start=(mt == 0),
                            stop=(mt == NMT - 1),
                        )

            # Estimated-time model for emission ordering: the PE executes
            # strictly in program order, so phase-C work for a tile must not
            # be emitted before its group's AllReduce has (by estimate)
            # landed; limited tiles per slot to avoid starving the fT
            # matmuls that feed the (bottleneck) ACT exp stream.
            TILE_T = 2.3
            CC_LAT = 11.0
            CC_GAP = 7.0
            est = 0.0
            cc_land = [None] * NG
            c_ready = []

            for g, gs in enumerate(GROUP_SIZES):
                scol = spool.tile([MTP, gs], f32, tag=f"scol{g}")
                for tl in range(gs):
                    mt = group_start[g] + tl
                    et = epool.tile([MTP, NLOC], bf16, tag="et")
                    ets[mt] = et
                    for hh in range(2):
                        ft = ftp.tile([MTP, 2 * CK], f32, tag="ft")
                        for jj in range(2):
                            j = 2 * hh + jj
                            nc.tensor.matmul(
                                ft[:, jj * CK : (jj + 1) * CK],
                                lhsT=phi_t[:, mt * MTP : (mt + 1) * MTP],
                                rhs=theta_t[:, j * CK : (j + 1) * CK],
                                start=True,
                                stop=True,
                            )
                        nc.scalar.activation(
                            out=et[:, hh * 2 * CK : (hh + 1) * 2 * CK],
                            in_=ft,
                            func=AF.Exp,
                        )
                    nc.vector.tensor_reduce(
                        out=scol[:, tl : tl + 1],
                        in_=et[:, :],
                        axis=AX.X,
                        op=ALU.add,
                    )
                    est += TILE_T
                    # dribble wgT matmuls into the early slots; once done,
                    # wgtp closes and the z accumulator takes its banks
                    if wgt_queue:
                        for _ in range(4):
                            if wgt_queue:
                                emit_wgt(wgt_queue.pop(0))
                        if not wgt_queue:
                            wgt_ctx.close()
                            zpp = ctx.enter_context(
                                tc.tile_pool(name="zpp", bufs=1, space="PSUM")
                            )
                            state["z"] = zpp.tile(
                                [MTP, NLOC // 2], f32, tag="z", name="z_ps"
                            )
                    else:
                        budget = 2 if (mt % 2) else 1
                        while budget and c_ready:
                            mt2 = c_ready[0]
                            land = cc_land[group_of[mt2]]
                            if land is not None and land <= est:
                                emit_c(c_ready.pop(0))
                                budget -= 1
                            else:
                                break
                # group complete: exchange softmax denominators
                cin = dpool.tile([MTP, gs], f32, tag=f"cin{g}")
                cout = dpool.tile([MTP, gs], f32, tag=f"cout{g}")
                nc.gpsimd.dma_start(out=cin, in_=scol)
                nc.gpsimd.collective_compute(
                    "AllReduce",
                    ALU.add,
                    replica_groups=REPLICA_GROUPS,
                    ins=[cin.opt()],
                    outs=[cout.opt()],
                )
                ssum = spool.tile([MTP, gs], f32, tag=f"ssum{g}")
                nc.sync.dma_start(out=ssum, in_=cout)
                srec = spool.tile([MTP, gs], f32, tag=f"srec{g}")
                nc.vector.reciprocal(out=srec, in_=ssum)
                srecs[g] = srec
                cc_land[g] = max(
                    est + CC_LAT,
                    (cc_land[g - 1] + CC_GAP) if g else 0.0,
                )
                c_ready.extend(range(group_start[g], group_start[g] + gs))

            while c_ready:
                emit_c(c_ready.pop(0))

            # ---------------- epilogue ----------------
            for jj in range(2):
                e2 = outp.tile([MTP, CK], f32, tag="e2")
                # (z + w_b) + supp in one DVE op
                nc.vector.scalar_tensor_tensor(
                    out=e2,
                    in0=state["z"][:, jj * CK : (jj + 1) * CK],
                    scalar=wb[:, :],
                    in1=supp_t[:, jj * CK : (jj + 1) * CK],
                    op0=ALU.add,
                    op1=ALU.add,
                )
                nc.sync.dma_start(
                    out=out[:, jj * CK : (jj + 1) * CK], in_=e2
                )

    nc.compile()
    return nc


def _get_nc():
    if "nc" not in _cache:
        _cache["nc"] = _build()
    return _cache["nc"]


def kernel(
    supp_feature,
    ref_feature,
    theta_w,
    theta_b,
    phi_w,
    phi_b,
    g_w,
    g_b,
    w_w,
    w_b,
    _trace=False,
):
    import ml_dtypes

    # run_bass_kernel_spmd imports antenv.axon_hooks when tracing is
    # requested (e.g. via BASS_TRACE in the environment); this container's
    # antenv stub lacks that module, so provide a no-op fallback.
    try:
        import antenv.axon_hooks  # noqa: F401
    except ImportError:
        import sys
        import types

        import antenv

        _mod = types.ModuleType("antenv.axon_hooks")
        _mod._hook = None
        _mod.get_axon_ntff_profile_hook = lambda: _mod._hook
        _mod.set_axon_ntff_profile_hook = lambda h: setattr(_mod, "_hook", h)
        sys.modules["antenv.axon_hooks"] = _mod
        antenv.axon_hooks = _mod

    from concourse.bass_utils import run_bass_kernel_spmd

    bf = ml_dtypes.bfloat16
    supp_feature = np.asarray(supp_feature, dtype=np.float32)
    ref_feature = np.asarray(ref_feature, dtype=np.float32)
    theta_w = np.asarray(theta_w, dtype=np.float32)
    theta_b = np.asarray(theta_b, dtype=np.float32)
    phi_w = np.asarray(phi_w, dtype=np.float32)
    phi_b = np.asarray(phi_b, dtype=np.float32)
    g_w = np.asarray(g_w, dtype=np.float32)
    g_b = np.asarray(g_b, dtype=np.float32)
    w_w = np.asarray(w_w, dtype=np.float32)
    w_b = np.asarray(w_b, dtype=np.float32)

    nc = _get_nc()

    supp2 = supp_feature.reshape(B, C, N)
    ref2 = ref_feature.reshape(B, C, N)
    # Fold the output 1x1 conv into g (weight-only transform):
    #   w_w @ (g_w @ ref + g_b) = (w_w@g_w) @ ref + (w_w@g_b)
    Wg = (w_w @ g_w).astype(np.float32)
    wgb = (w_w @ g_b).astype(np.float32)
    wg_aug = np.ascontiguousarray(
        np.concatenate([Wg.T, wgb[None, :]], axis=0).astype(bf)
    )
    theta_wTh = np.ascontiguousarray(theta_w.T.astype(bf))
    phi_wTh = np.ascontiguousarray(phi_w.T.astype(bf))
    w_bc2 = np.ascontiguousarray(
        np.concatenate([w_b, w_b]).reshape(MTP, 1).astype(np.float32)
    )

    in_maps = []
    for core in range(NCORES):
        b, h = core // 2, core % 2
        ref_aug = np.ascontiguousarray(
            np.concatenate(
                [ref2[b], np.ones((1, N), np.float32)], axis=0
            ).astype(bf)
        )
        sloc = supp2[b, :, h * NLOC : (h + 1) * NLOC]
        # repack [64, 2048] -> [128, 1024]: rows 64:128 hold n-cols 1024:2048
        supp_rp = np.ascontiguousarray(
            sloc.reshape(C, 2, NLOC // 2).transpose(1, 0, 2).reshape(MTP, NLOC // 2)
        )
        in_maps.append(
            {
                "supp_rp": supp_rp,
                "supp_b": np.ascontiguousarray(sloc.astype(bf)),
                "ref_aug": ref_aug,
                "theta_wT": theta_wTh,
                "theta_bc": np.ascontiguousarray(theta_b.reshape(CI, 1)),
                "phi_wT": phi_wTh,
                "phi_bc": np.ascontiguousarray(phi_b.reshape(CI, 1)),
                "wg_aug": wg_aug,
                "w_bc2": w_bc2,
            }
        )

    res = run_bass_kernel_spmd(
        nc, in_maps, list(range(NCORES)), trace=_trace
    )
    if _trace:
        _cache["last_exec_time_ns"] = res.exec_time_ns
        _cache["last_results"] = res

    z = np.empty((B, C, N), dtype=np.float32)
    for core in range(NCORES):
        b, h = core // 2, core % 2
        o = res.results[core]["out"]  # [128, 1024]
        z[b, :, h * NLOC : h * NLOC + NLOC // 2] = o[0:C]
        z[b, :, h * NLOC + NLOC // 2 : (h + 1) * NLOC] = o[C:MTP]
    return z.reshape(B, C, H, W)
